# revision 8
# baseline (speedup 1.0000x reference)
"""Dense multi-head attention (DotProductAttention) for Trainium2, 8-core SPMD.

Full inputs: query/key/value [b=2, s=2048, nh=32, hn=64] fp32.
Sharding: b*nh = 64 head-units split across 8 cores (8 units/core),
each core computes full attention for its units, no cross-core comms.

Per-core dataflow (per head-unit u, per q-granule g of 1024):
  qT, kT : [64, 2048] SBUF (hn on partitions; host pre-transposed),
           cast to fp32r (TF32-like, 1 PE cycle/row vs 4 for fp32)
  S^T    : [k-tile=128, 1024] = kT-tile^T @ qT chunk, PSUM ping-pong
  exp    : ScalarE Exp(scale=1/sqrt(hn)) PSUM -> SBUF fp32r P^T.
           No max subtraction: scores ~ N(0,1), |s| < ~6, exp is safe
           in fp32 and softmax is shift-invariant.
  PV     : ctx~T [65, 1024] += V~[k-tile]^T @ P^T accumulated over 16
           k-tiles in PSUM; V~ has a ones column so row 64 = sum_k P
           (the softmax denominator).
  norm   : evict ctx~T to SBUF, PE-transpose back to PSUM as [128, 8, 65]
           (q on partitions), then the denominator is a per-partition
           scalar: reciprocal + tensor_scalar_mul.
  out    : [1024, 64] natural layout -> DRAM.
"""

import numpy as np
from contextlib import ExitStack

import concourse.bass as bass
import concourse.tile as tile
from concourse import bacc, mybir
from concourse.bass_utils import run_bass_kernel_spmd
from concourse.masks import make_identity

F32 = mybir.dt.float32
F32R = mybir.dt.float32r
EXP = mybir.ActivationFunctionType.Exp

N_CORES = 8


def build_attention_nc(n_units=8, sq=2048, sk=2048, hn=64, q_gran=1024,
                       num_devices=N_CORES):
    """Build + compile the per-core bass program."""
    assert sk % 128 == 0 and sq % q_gran == 0 and q_gran % 512 == 0
    n_ktiles = sk // 128
    n_qgran = sq // q_gran
    n_chunk = q_gran // 512
    n_qsub = q_gran // 128
    inv_norm = 1.0 / float(np.sqrt(np.float32(hn)))

    nc = bacc.Bacc("TRN2", target_bir_lowering=False, debug=False,
                   num_devices=num_devices)

    qT = nc.dram_tensor("qT", [n_units, hn, sq], F32, kind="ExternalInput").ap()
    kT = nc.dram_tensor("kT", [n_units, hn, sk], F32, kind="ExternalInput").ap()
    v = nc.dram_tensor("v", [n_units, sk, hn], F32, kind="ExternalInput").ap()
    out = nc.dram_tensor("out", [n_units, sq, hn], F32,
                         kind="ExternalOutput").ap()

    with tile.TileContext(nc) as tc, ExitStack() as ctx:
        const_pool = ctx.enter_context(tc.tile_pool(name="const", bufs=1))
        qk_pool = ctx.enter_context(tc.tile_pool(name="qk", bufs=2))
        v_pool = ctx.enter_context(tc.tile_pool(name="v", bufs=2))
        p_pool = ctx.enter_context(tc.tile_pool(name="p", bufs=3))
        o_pool = ctx.enter_context(tc.tile_pool(name="o", bufs=3))
        sm_pool = ctx.enter_context(tc.tile_pool(name="sm", bufs=3))
        stage_pool = ctx.enter_context(
            tc.tile_pool(name="stage", bufs=2, space="PSUM"))
        ctxp_pool = ctx.enter_context(
            tc.tile_pool(name="ctxp", bufs=2, space="PSUM"))

        ident = const_pool.tile([hn + 1, hn + 1], F32)
        make_identity(nc, ident)

        for u in range(n_units):
            # ---- load operands; fp32r matmul inputs must be produced by
            # a rounding engine op, so DMA raw fp32 then DVE cast-copy ----
            qT_raw = qk_pool.tile([hn, sq], F32, tag="qTr")
            nc.sync.dma_start(qT_raw[:], qT[u])
            qT_sb = qk_pool.tile([hn, sq], F32R, tag="qT")
            nc.vector.tensor_copy(qT_sb[:], qT_raw[:])
            kT_raw = qk_pool.tile([hn, sk], F32, tag="kTr")
            nc.sync.dma_start(kT_raw[:], kT[u])
            kT_sb = qk_pool.tile([hn, sk], F32R, tag="kT")
            nc.vector.tensor_copy(kT_sb[:], kT_raw[:])
            # v tiles with an appended ones column (fp32 build + cast:
            # memset directly on an f32r tile is invalid ISA)
            v_raw = v_pool.tile([128, n_ktiles, hn + 1], F32, tag="vr")
            nc.sync.dma_start(
                v_raw[:, :, 0:hn], v[u].rearrange("(t p) h -> p t h", p=128))
            nc.vector.memset(v_raw[:, :, hn:hn + 1], 1.0)
            v_sb = v_pool.tile([128, n_ktiles, hn + 1], F32R, tag="v")
            nc.vector.tensor_copy(v_sb[:], v_raw[:])

            for g in range(n_qgran):
                ctx_ps = ctxp_pool.tile([hn + 1, q_gran], F32, tag="ctx")
                for i in range(n_ktiles):
                    stage = stage_pool.tile([128, q_gran], F32, tag="stage")
                    lhsT = kT_sb[:, i * 128:(i + 1) * 128]
                    for c in range(n_chunk):
                        q0 = g * q_gran + c * 512
                        nc.tensor.matmul(
                            stage[:, c * 512:(c + 1) * 512],
                            lhsT,
                            qT_sb[:, q0:q0 + 512],
                            start=True, stop=True)
                    pT = p_pool.tile([128, q_gran], F32R, tag="pT")
                    nc.scalar.activation(pT[:], stage[:], EXP, scale=inv_norm)
                    vT = v_sb[:, i, :]
                    for c in range(n_chunk):
                        nc.tensor.matmul(
                            ctx_ps[:, c * 512:(c + 1) * 512],
                            vT,
                            pT[:, c * 512:(c + 1) * 512],
                            start=(i == 0), stop=(i == n_ktiles - 1))

                # ---- normalize: transpose ctx~T so the denominator is a
                # per-partition scalar, then reciprocal + scalar-mul ----
                ctx_sb = o_pool.tile([hn + 1, q_gran], F32, tag="ctxsb")
                nc.vector.tensor_copy(ctx_sb[:], ctx_ps[:])
                # pad subtile stride to 128 floats: a matmul (transpose)
                # output must not cross a PSUM bank boundary
                ctt = ctxp_pool.tile([128, n_qsub, 128], F32, tag="ctx")
                for j in range(n_qsub):
                    nc.tensor.transpose(
                        ctt[:, j, 0:hn + 1], ctx_sb[:, j * 128:(j + 1) * 128],
                        ident[:])
                recipc = sm_pool.tile([128, n_qsub], F32, tag="recipc")
                nc.vector.reciprocal(recipc[:], ctt[:, :, hn])
                o_sb = o_pool.tile([128, n_qsub, hn], F32, tag="o")
                for j in range(n_qsub):
                    nc.vector.tensor_scalar_mul(
                        o_sb[:, j, :], ctt[:, j, 0:hn],
                        recipc[:, j:j + 1])
                nc.sync.dma_start(
                    out[u].rearrange("(gg j p) h -> p (gg j) h",
                                     p=128, j=n_qsub)[:, g * n_qsub:
                                                      (g + 1) * n_qsub, :],
                    o_sb[:])

    nc.compile()
    return nc


_CACHE = {}


def _get_nc():
    if "nc" not in _CACHE:
        _CACHE["nc"] = build_attention_nc()
    return _CACHE["nc"]


def kernel(query, key, value):
    b, sq, nh, hn = query.shape
    assert (b, sq, nh, hn) == (2, 2048, 32, 64)
    nu = b * nh
    per = nu // N_CORES

    qT = np.ascontiguousarray(
        query.transpose(0, 2, 3, 1).reshape(nu, hn, sq)).astype(np.float32)
    kT = np.ascontiguousarray(
        key.transpose(0, 2, 3, 1).reshape(nu, hn, sq)).astype(np.float32)
    vv = np.ascontiguousarray(
        value.transpose(0, 2, 1, 3).reshape(nu, sq, hn)).astype(np.float32)

    nc = _get_nc()
    in_maps = [
        {"qT": qT[c * per:(c + 1) * per],
         "kT": kT[c * per:(c + 1) * per],
         "v": vv[c * per:(c + 1) * per]}
        for c in range(N_CORES)
    ]
    res = run_bass_kernel_spmd(nc, in_maps, list(range(N_CORES)))
    ctxo = np.concatenate([res.results[c]["out"] for c in range(N_CORES)],
                          axis=0)  # [nu, sq, hn]
    outp = ctxo.reshape(b, nh, sq, hn).transpose(0, 2, 1, 3)
    return np.ascontiguousarray(outp.reshape(b, sq, nh * hn)).astype(np.float32)


# revision 11
# speedup vs baseline: 4920.1117x; 4920.1117x over previous
"""Dense multi-head attention (DotProductAttention) for Trainium2, 8-core SPMD.

Full inputs: query/key/value [b=2, s=2048, nh=32, hn=64] fp32.
Sharding: b*nh = 64 head-units split across 8 cores (8 units/core),
each core computes full attention for its units, no cross-core comms.

Per-core dataflow (per head-unit u, per q-granule g of 1024):
  qT, kT : [64, 2048] SBUF (hn on partitions; host pre-transposed),
           cast to fp32r (TF32-like, 1 PE cycle/row vs 4 for fp32)
  S^T    : [k-tile=128, 1024] = kT-tile^T @ qT chunk, PSUM ping-pong
  exp    : ScalarE Exp(scale=1/sqrt(hn)) PSUM -> SBUF fp32r P^T.
           No max subtraction: scores ~ N(0,1), |s| < ~6, exp is safe
           in fp32 and softmax is shift-invariant.
  PV     : ctx~T [65, 1024] += V~[k-tile]^T @ P^T accumulated over 16
           k-tiles in PSUM; V~ has a ones column so row 64 = sum_k P
           (the softmax denominator).
  norm   : evict ctx~T to SBUF, PE-transpose back to PSUM as [128, 8, 65]
           (q on partitions), then the denominator is a per-partition
           scalar: reciprocal + tensor_scalar_mul.
  out    : [1024, 64] natural layout -> DRAM.
"""

import numpy as np
from contextlib import ExitStack

import concourse.bass as bass
import concourse.tile as tile
from concourse import bacc, mybir
from concourse.bass_utils import run_bass_kernel_spmd
from concourse.masks import make_identity

F32 = mybir.dt.float32
F32R = mybir.dt.float32r
EXP = mybir.ActivationFunctionType.Exp

N_CORES = 8


def build_attention_nc(n_units=8, sq=2048, sk=2048, hn=64, q_gran=1024,
                       num_devices=N_CORES, loop_iters=1):
    """Build + compile the per-core bass program.

    loop_iters > 1 wraps the body in an on-device repeat loop (for
    benchmarking via the wall-clock slope between two loop counts)."""
    assert sk % 128 == 0 and sq % q_gran == 0 and q_gran % 512 == 0
    n_ktiles = sk // 128
    n_qgran = sq // q_gran
    n_chunk = q_gran // 512
    n_qsub = q_gran // 128
    inv_norm = 1.0 / float(np.sqrt(np.float32(hn)))

    nc = bacc.Bacc("TRN2", target_bir_lowering=False, debug=False,
                   num_devices=num_devices)

    qT = nc.dram_tensor("qT", [n_units, hn, sq], F32, kind="ExternalInput").ap()
    kT = nc.dram_tensor("kT", [n_units, hn, sk], F32, kind="ExternalInput").ap()
    v = nc.dram_tensor("v", [n_units, sk, hn], F32, kind="ExternalInput").ap()
    out = nc.dram_tensor("out", [n_units, sq, hn], F32,
                         kind="ExternalOutput").ap()

    with tile.TileContext(nc) as tc, ExitStack() as ctx:
        const_pool = ctx.enter_context(tc.tile_pool(name="const", bufs=1))
        qk_pool = ctx.enter_context(tc.tile_pool(name="qk", bufs=2))
        v_pool = ctx.enter_context(tc.tile_pool(name="v", bufs=2))
        p_pool = ctx.enter_context(tc.tile_pool(name="p", bufs=3))
        o_pool = ctx.enter_context(tc.tile_pool(name="o", bufs=3))
        sm_pool = ctx.enter_context(tc.tile_pool(name="sm", bufs=3))
        stage_pool = ctx.enter_context(
            tc.tile_pool(name="stage", bufs=2, space="PSUM"))
        ctxp_pool = ctx.enter_context(
            tc.tile_pool(name="ctxp", bufs=2, space="PSUM"))

        ident = const_pool.tile([hn + 1, hn + 1], F32)
        make_identity(nc, ident)

        loop_cm = tc.For_i(0, loop_iters, 1) if loop_iters > 1 else None
        if loop_cm is not None:
            loop_cm.__enter__()

        for u in range(n_units):
            # ---- load operands; fp32r matmul inputs must be produced by
            # a rounding engine op, so DMA raw fp32 then DVE cast-copy ----
            qT_raw = qk_pool.tile([hn, sq], F32, tag="qTr")
            nc.sync.dma_start(qT_raw[:], qT[u])
            qT_sb = qk_pool.tile([hn, sq], F32R, tag="qT")
            nc.vector.tensor_copy(qT_sb[:], qT_raw[:])
            kT_raw = qk_pool.tile([hn, sk], F32, tag="kTr")
            nc.sync.dma_start(kT_raw[:], kT[u])
            kT_sb = qk_pool.tile([hn, sk], F32R, tag="kT")
            nc.vector.tensor_copy(kT_sb[:], kT_raw[:])
            # v tiles with an appended ones column (fp32 build + cast:
            # memset directly on an f32r tile is invalid ISA)
            v_raw = v_pool.tile([128, n_ktiles, hn + 1], F32, tag="vr")
            nc.sync.dma_start(
                v_raw[:, :, 0:hn], v[u].rearrange("(t p) h -> p t h", p=128))
            nc.vector.memset(v_raw[:, :, hn:hn + 1], 1.0)
            v_sb = v_pool.tile([128, n_ktiles, hn + 1], F32R, tag="v")
            nc.vector.tensor_copy(v_sb[:], v_raw[:])

            for g in range(n_qgran):
                ctx_ps = ctxp_pool.tile([hn + 1, q_gran], F32, tag="ctx")
                for i in range(n_ktiles):
                    stage = stage_pool.tile([128, q_gran], F32, tag="stage")
                    lhsT = kT_sb[:, i * 128:(i + 1) * 128]
                    for c in range(n_chunk):
                        q0 = g * q_gran + c * 512
                        nc.tensor.matmul(
                            stage[:, c * 512:(c + 1) * 512],
                            lhsT,
                            qT_sb[:, q0:q0 + 512],
                            start=True, stop=True)
                    pT = p_pool.tile([128, q_gran], F32R, tag="pT")
                    nc.scalar.activation(pT[:], stage[:], EXP, scale=inv_norm)
                    vT = v_sb[:, i, :]
                    for c in range(n_chunk):
                        nc.tensor.matmul(
                            ctx_ps[:, c * 512:(c + 1) * 512],
                            vT,
                            pT[:, c * 512:(c + 1) * 512],
                            start=(i == 0), stop=(i == n_ktiles - 1))

                # ---- normalize: transpose ctx~T so the denominator is a
                # per-partition scalar, then reciprocal + scalar-mul ----
                ctx_sb = o_pool.tile([hn + 1, q_gran], F32, tag="ctxsb")
                nc.vector.tensor_copy(ctx_sb[:], ctx_ps[:])
                # pad subtile stride to 128 floats: a matmul (transpose)
                # output must not cross a PSUM bank boundary
                ctt = ctxp_pool.tile([128, n_qsub, 128], F32, tag="ctx")
                for j in range(n_qsub):
                    nc.tensor.transpose(
                        ctt[:, j, 0:hn + 1], ctx_sb[:, j * 128:(j + 1) * 128],
                        ident[:])
                recipc = sm_pool.tile([128, n_qsub], F32, tag="recipc")
                nc.vector.reciprocal(recipc[:], ctt[:, :, hn])
                o_sb = o_pool.tile([128, n_qsub, hn], F32, tag="o")
                for j in range(n_qsub):
                    nc.vector.tensor_scalar_mul(
                        o_sb[:, j, :], ctt[:, j, 0:hn],
                        recipc[:, j:j + 1])
                nc.sync.dma_start(
                    out[u].rearrange("(gg j p) h -> p (gg j) h",
                                     p=128, j=n_qsub)[:, g * n_qsub:
                                                      (g + 1) * n_qsub, :],
                    o_sb[:])

        if loop_cm is not None:
            loop_cm.__exit__(None, None, None)

    nc.compile()
    return nc


_CACHE = {}


def _get_nc():
    if "nc" not in _CACHE:
        _CACHE["nc"] = build_attention_nc()
    return _CACHE["nc"]


def kernel(query, key, value):
    b, sq, nh, hn = query.shape
    assert (b, sq, nh, hn) == (2, 2048, 32, 64)
    nu = b * nh
    per = nu // N_CORES

    qT = np.ascontiguousarray(
        query.transpose(0, 2, 3, 1).reshape(nu, hn, sq)).astype(np.float32)
    kT = np.ascontiguousarray(
        key.transpose(0, 2, 3, 1).reshape(nu, hn, sq)).astype(np.float32)
    vv = np.ascontiguousarray(
        value.transpose(0, 2, 1, 3).reshape(nu, sq, hn)).astype(np.float32)

    nc = _get_nc()
    in_maps = [
        {"qT": qT[c * per:(c + 1) * per],
         "kT": kT[c * per:(c + 1) * per],
         "v": vv[c * per:(c + 1) * per]}
        for c in range(N_CORES)
    ]
    res = run_bass_kernel_spmd(nc, in_maps, list(range(N_CORES)))
    ctxo = np.concatenate([res.results[c]["out"] for c in range(N_CORES)],
                          axis=0)  # [nu, sq, hn]
    outp = ctxo.reshape(b, nh, sq, hn).transpose(0, 2, 1, 3)
    return np.ascontiguousarray(outp.reshape(b, sq, nh * hn)).astype(np.float32)
